# revision 22
# baseline (speedup 1.0000x reference)
"""DOSAConLoss Trainium2 kernel.

result = mean(base) * mean(1 + ALPHA * density)
       = mean(base) * (1 + ALPHA * (N/1024) / max_hist)

since sum(hist) == N exactly (every box center lands in one bin).

Per core (8-way data parallel over N): compute
  - per-partition partial sums of base  (acc_out [128, n_tiles])
  - radix-packed 32x32 histogram of target box centers (hist_out)
Host combines: sums acc, decodes + sums hists, applies the scalar formula.

Inputs are staged PLANAR ([4, NB]: x,y,w,h planes) so every DVE read is
dense (the original xywh interleave costs ~2x on strided reads).

Math rewrite (validated vs reference in fp64/fp32):
  dx=x1-x2, W=w1+w2, dW=w1-w2 (same for y/h)
  iw4 = relu(W - max(|2dx|,|dW|)) = 2*iw ; inter4 = iw4*ih4 = 4*inter
  u = a1+a2 - inter4/4 (+eps)  ; iou = inter4 * 0.25/(u+eps)
  cw2 = W + mx = 2*cw ; c24 = cw2^2+ch2^2 = 4*c2 ; rho4 = (2dx)^2+(2dy)^2
  rho2/c2 == rho4/c24
  atan(w/h) = pi/4 + atan((w-h)/(w+h))  [exact identity, arg in (-1,1)]
    -> theta2-theta1 = atan(q2)-atan(q1), q=(w-h)/(w+h)
  v = ((th2-th1)*2/pi)^2 ; ciou = iou - rho4/c24 - v^2/(v-iou+1+eps)
  base = (1-ciou)^3 / (w2*h2 + 1e-7)
Reciprocals via exp(-ln(x)) (ACT Reciprocal is disallowed in bass).

Histogram (radix-packed 2:1 on both axes, diagonal-block matmuls):
  hx = trickfloor(16x) via magic-number RNE rounding; px = [16x+0.5-1 >= hx]
  (x-bin = 2*hx+px), same for y. Weights wx = 64^px, wy = 4096^py.
  One-hot planes (plane-major, dense DVE writes):
    ohx[p, i*T+t] = [hx[p,t]==i]*wx ; ohy[p, m*T+t] = [hy==m]*wy
  Eight box-columns merge into ONE [128,128]x[128,128] matmul: operand APs
  enumerate free dims as (ts=t%8 inner, bin outer), so psum[(tr,m),(ts,i)]
  sums ohy_t[k,m]*ohx_t[k,i] -- the 8 diagonal 16x16 blocks (tr==ts) are the
  true per-column outer products (accumulated over the tile via PSUM); the
  off-diagonal cross-column blocks are garbage and never read. 64 matmuls
  per tile instead of 512 LDWEIGHTS+MATMUL pairs per tile.
  Each psum cell accumulates 64^(px+2py) per box: base-64 digits = counts of
  the four (py,px) sub-bins; digit max ~23 < 64 for uniform inputs.
  Host decodes digits, exactly relocating the few fp-tie boxes where the
  device trick-bin differs from floor.
"""

import numpy as np

import concourse.bass as bass
import concourse.bacc as bacc
import concourse.mybir as mybir
import concourse.tile as tile
from concourse import bass_utils

# The act-table-load chooser picks the first set containing each function,
# which puts Ln in `natural_log` and Exp in `exp_and_others`, forcing a
# ~2.7us table switch at every Ln->Exp pair (we use exp(-ln(x)) for all
# reciprocals). Hide Ln/Exp from the single-function sets so the chooser
# lands on `natural_log_exp_and_others` (set ids keep their act_info.json
# positions; only membership is masked).
_orig_get_act_tables = bacc.get_activation_tables


def _patched_get_act_tables(arch):
    t = {k: set(v) for k, v in _orig_get_act_tables(arch).items()}
    t.get("natural_log", set()).discard(mybir.ActivationFunctionType.Ln)
    t.get("exp_and_others", set()).discard(mybir.ActivationFunctionType.Exp)
    t.get("exp_and_friends", set()).discard(mybir.ActivationFunctionType.Exp)
    return t


bacc.get_activation_tables = _patched_get_act_tables

F32 = mybir.dt.float32
BF16 = mybir.dt.bfloat16
AF = mybir.ActivationFunctionType
OP = mybir.AluOpType

GRID = 32
ALPHA = 1.5
EPS = 1e-7
PI = float(np.pi)
MAGIC = float(2 ** 23)

N_CORES = 8
N_TOTAL = 4_000_000
NB_CORE = 524_288            # padded boxes per core: 128 * 4096
# pad boxes: pred == targ -> base ~1e-21; x=y=0 -> trick-bin -1 -> never
# deposited in the histogram (no host fixup needed).
PAD_BOX = (0.0, 0.0, 1.0, 1.0)

# GPSIMD offload set for 2-input tensor_tensor ops (tune via profile)
# (POOL TensorTensor float ops: only add/subtract/mult are ISA-legal)
GPS_OPS = {"asum", "cw2", "ch2", "c24", "rho4", "s2", "s1"}


def build_nc(NB, T=512, gps=True):
    """Build the per-core Bass program. NB must equal n_tiles*128*T."""
    n_tiles = NB // (128 * T)
    assert NB == n_tiles * 128 * T
    assert T % 8 == 0

    nc = bacc.Bacc("TRN2", target_bir_lowering=False, debug=False)
    pred_d = nc.dram_tensor("pred_boxes", [4, NB], F32, kind="ExternalInput")
    targ_d = nc.dram_tensor("target_boxes", [4, NB], F32, kind="ExternalInput")
    acc_d = nc.dram_tensor("acc_out", [128, n_tiles], F32, kind="ExternalOutput")
    hist_d = nc.dram_tensor("hist_out", [128, 128 * n_tiles], F32, kind="ExternalOutput")

    # planar: channel c of tile n -> [128, T] dense
    pred_v = pred_d.ap().rearrange("c (n p t) -> n p c t", p=128, t=T)
    targ_v = targ_d.ap().rearrange("c (n p t) -> n p c t", p=128, t=T)

    def eng(name):
        return nc.gpsimd if (gps and name in GPS_OPS) else nc.vector

    with tile.TileContext(nc) as tc:
        with (
            tc.tile_pool(name="inp", bufs=3) as inp,
            tc.tile_pool(name="tmp", bufs=2) as tmp,
            tc.tile_pool(name="ohp", bufs=2) as ohp,
            tc.tile_pool(name="cst", bufs=1) as cst,
            tc.tile_pool(name="psp", bufs=2, space="PSUM") as psp,
        ):
            bias_tiles = {}

            def bias_ap(val):
                if val not in bias_tiles:
                    t = cst.tile([128, 1], F32, name=f"bias{len(bias_tiles)}")
                    nc.vector.memset(t[:], val)
                    bias_tiles[val] = t[:]
                return bias_tiles[val]

            acc_sb = cst.tile([128, n_tiles], F32)
            hist_sb = cst.tile([128, 128 * n_tiles], F32)

            # Temp slot allocator: long-lived temps get dedicated tags;
            # short-lived ones rotate through NGEN generic tags (bufs=2 each,
            # Tile inserts WAR deps on slot reuse). Max temp lifetime must be
            # < 2*NGEN generic allocations.
            NGEN = 12
            DEDICATED = {"W", "H", "a2t", "iou", "term1", "vv"}
            gen_counter = [0]

            for n in range(n_tiles):
                pt = inp.tile([128, 4 * T], F32, tag="pred")
                tt = inp.tile([128, 4 * T], F32, tag="targ")
                p3 = pt.rearrange("p (c t) -> p c t", c=4)
                t3 = tt.rearrange("p (c t) -> p c t", c=4)
                nc.sync.dma_start(p3, pred_v[n])
                nc.sync.dma_start(t3, targ_v[n])
                x1, y1, w1, h1 = p3[:, 0], p3[:, 1], p3[:, 2], p3[:, 3]
                x2, y2, w2, h2 = t3[:, 0], t3[:, 1], t3[:, 2], t3[:, 3]

                def t_(tag, dt=F32):
                    if tag in DEDICATED:
                        return tmp.tile([128, T], dt, tag=tag, name=tag)[:]
                    i = gen_counter[0] % NGEN
                    gen_counter[0] += 1
                    return tmp.tile([128, T], dt, tag=f"g{i}", name=tag)[:]

                # ---- histogram prep first (depends only on targ x/y) ----
                # hx = trickfloor(16x): t1 = RNE(16x + 0.5 + 2^23);
                # hx = t1 - (2^23+1). Pair parity px = [16x+0.5 - hx > 1] via
                # exact arithmetic (relational-op1 stt runs ~2.5x slower):
                # r = relu(s-1), px = min(r*2^24, 1) -- all steps f32-exact and
                # host-replicable. Combined weight wxy = 64^px * 4096^py rides
                # on the y-side one-hot so the x-side is single-input is_equal.
                # Ties / out-of-range corrected host-side (see _decode_hists).
                zx, zy, hxf, hyf = t_("zx"), t_("zy"), t_("hxf"), t_("hyf")
                sx, sy, rx, ry = t_("sx"), t_("sy"), t_("rx"), t_("ry")
                hx = tmp.tile([128, T], BF16, tag="hx", name="hx")[:]
                px = tmp.tile([128, T], BF16, tag="px", name="px")[:]
                wx = tmp.tile([128, T], BF16, tag="wx", name="wx")[:]
                hy = tmp.tile([128, T], BF16, tag="hy", name="hy")[:]
                py = tmp.tile([128, T], BF16, tag="py", name="py")[:]
                wy = tmp.tile([128, T], BF16, tag="wy", name="wy")[:]
                wxy = tmp.tile([128, T], BF16, tag="wxy", name="wxy")[:]
                nc.vector.tensor_scalar(zx, x2, 16.0, 0.5, OP.mult, OP.add)
                nc.vector.tensor_scalar(hxf, zx, MAGIC, MAGIC + 1.0, OP.add, OP.subtract)
                nc.vector.tensor_scalar(hx, zx, MAGIC, MAGIC + 1.0, OP.add, OP.subtract)
                nc.vector.tensor_tensor(sx, zx, hxf, OP.subtract)
                nc.scalar.activation(rx, sx, AF.Relu, bias=bias_ap(-1.0))
                nc.vector.tensor_scalar(zy, y2, 16.0, 0.5, OP.mult, OP.add)
                nc.vector.tensor_scalar(hyf, zy, MAGIC, MAGIC + 1.0, OP.add, OP.subtract)
                nc.vector.tensor_scalar(hy, zy, MAGIC, MAGIC + 1.0, OP.add, OP.subtract)
                nc.vector.tensor_tensor(sy, zy, hyf, OP.subtract)
                nc.scalar.activation(ry, sy, AF.Relu, bias=bias_ap(-1.0))

                # ---- first-level geometry (DVE, dep-free) ----
                dx, dy = t_("dx"), t_("dy")
                W, dW, H, dH = t_("W"), t_("dW"), t_("H"), t_("dH")
                nc.vector.tensor_tensor(dx, x1, x2, OP.subtract)
                nc.vector.tensor_tensor(dy, y1, y2, OP.subtract)
                nc.vector.tensor_tensor(W, w1, w2, OP.add)
                nc.vector.tensor_tensor(dW, w1, w2, OP.subtract)
                nc.vector.tensor_tensor(H, h1, h2, OP.add)
                nc.vector.tensor_tensor(dH, h1, h2, OP.subtract)
                a2t, a1t, asum = t_("a2t"), t_("a1t"), t_("asum")
                nc.vector.tensor_tensor(a2t, w2, h2, OP.mult)
                nc.vector.tensor_tensor(a1t, w1, h1, OP.mult)
                eng("asum").tensor_tensor(asum, a1t, a2t, OP.add)

                adx, ady, adW, adH = t_("adx"), t_("ady"), t_("adW"), t_("adH")
                nc.scalar.activation(adx, dx, AF.Abs, scale=2.0)
                nc.scalar.activation(ady, dy, AF.Abs, scale=2.0)
                nc.scalar.activation(adW, dW, AF.Abs)
                nc.scalar.activation(adH, dH, AF.Abs)

                nc.vector.tensor_scalar(px, rx, 16777216.0, 1.0, OP.mult, OP.min)
                nc.vector.tensor_scalar(py, ry, 16777216.0, 1.0, OP.mult, OP.min)
                nc.scalar.activation(wx, px, AF.Copy, scale=63.0, bias=1.0)
                nc.scalar.activation(wy, py, AF.Copy, scale=4095.0, bias=1.0)
                nc.vector.tensor_tensor(wxy, wx, wy, OP.mult)

                # ---- one-hot build, GROUP-BLOCKED layout ----
                # ohx[p, g*128 + i*8 + ts] = [hx[p, 8g+ts]==i] (plain 0/1);
                # ohy[...] = [hy==i]*wxy. Per bin the write AP is
                # (g: stride 128) x (ts: 8 contiguous) -- 16B runs, near-dense
                # for the DVE write port -- while each group's matmul operand
                # [128, 128] is fully contiguous. The 32 plane ops are emitted
                # interleaved into the main chain as DVE filler in front of
                # cross-engine dependencies (ACT/GPS results arrive ~1-5us
                # late; DVE queues are strict FIFO so independent work must
                # sit BEFORE the dependent op to hide the latency).
                ohx = ohp.tile([128, 16 * T], BF16, tag="ohx", name="ohx")
                ohy = ohp.tile([128, 16 * T], BF16, tag="ohy", name="ohy")
                vx = ohx.rearrange("p (g i ts) -> p i g ts", i=16, ts=8)
                vy = ohy.rearrange("p (g i ts) -> p i g ts", i=16, ts=8)
                hx8 = hx.rearrange("p (g ts) -> p g ts", ts=8)
                hy8 = hy.rearrange("p (g ts) -> p g ts", ts=8)
                wxy8 = wxy.rearrange("p (g ts) -> p g ts", ts=8)
                planes = []
                for i in range(16):
                    planes.append((vx, i, None))
                    planes.append((vy, i, wxy8))
                plane_i = [0]

                def fill(k):
                    for _ in range(k):
                        if plane_i[0] >= len(planes):
                            return
                        v, i, w = planes[plane_i[0]]
                        plane_i[0] += 1
                        if w is None:
                            nc.vector.tensor_scalar(
                                v[:, i], hx8, float(i), None, OP.is_equal)
                        else:
                            nc.vector.scalar_tensor_tensor(
                                v[:, i], hy8, float(i), w, OP.is_equal, OP.mult)

                fill(6)   # x-planes while ACT computes the abs block
                mx, my = t_("mx"), t_("my")
                nc.vector.tensor_tensor(mx, adx, adW, OP.max)
                nc.vector.tensor_tensor(my, ady, adH, OP.max)

                iw4, ih4, ihc, inter4 = t_("iw4"), t_("ih4"), t_("ihc"), t_("inter4")
                nc.vector.scalar_tensor_tensor(iw4, mx, -1.0, W, OP.mult, OP.add)
                nc.vector.scalar_tensor_tensor(ih4, my, -1.0, H, OP.mult, OP.add)
                nc.vector.tensor_scalar(ihc, ih4, 0.0, None, OP.max)
                cw2, ch2 = t_("cw2"), t_("ch2")
                eng("cw2").tensor_tensor(cw2, W, mx, OP.add)
                eng("ch2").tensor_tensor(ch2, H, my, OP.add)
                fill(3)
                nc.vector.scalar_tensor_tensor(inter4, iw4, 0.0, ihc, OP.max, OP.mult)
                u = t_("u")
                nc.vector.scalar_tensor_tensor(u, inter4, -0.25, asum, OP.mult, OP.add)
                lnu, r_u = t_("lnu"), t_("r_u")
                nc.scalar.activation(lnu, u, AF.Ln, scale=4.0, bias=bias_ap(4 * EPS))
                nc.scalar.activation(r_u, lnu, AF.Exp, scale=-1.0)
                scw, sch, sdx, sdy = t_("scw"), t_("sch"), t_("sdx"), t_("sdy")
                nc.scalar.activation(scw, cw2, AF.Square)
                nc.scalar.activation(sch, ch2, AF.Square)
                nc.scalar.activation(sdx, adx, AF.Square)
                nc.scalar.activation(sdy, ady, AF.Square)
                c24, rho4 = t_("c24"), t_("rho4")
                eng("c24").tensor_tensor(c24, scw, sch, OP.add)
                eng("rho4").tensor_tensor(rho4, sdx, sdy, OP.add)
                lnc, r_c = t_("lnc"), t_("r_c")
                nc.scalar.activation(lnc, c24, AF.Ln, bias=bias_ap(4 * EPS))
                nc.scalar.activation(r_c, lnc, AF.Exp, scale=-1.0)
                fill(4)
                iou = t_("iou")
                nc.vector.tensor_tensor(iou, inter4, r_u, OP.mult)

                # atan(w/h) = pi/4 + atan((w-h)/(w+h)); pi/4 cancels in the
                # difference, so dat = atan(q2) - atan(q1) directly.
                s2, d2, s1, d1 = t_("s2"), t_("d2"), t_("s1"), t_("d1")
                eng("s2").tensor_tensor(s2, w2, h2, OP.add)
                nc.vector.tensor_tensor(d2, w2, h2, OP.subtract)
                eng("s1").tensor_tensor(s1, w1, h1, OP.add)
                nc.vector.tensor_tensor(d1, w1, h1, OP.subtract)
                ls2, r2s, ls1, r1s = t_("ls2"), t_("r2s"), t_("ls1"), t_("r1s")
                nc.scalar.activation(ls2, s2, AF.Ln)
                nc.scalar.activation(r2s, ls2, AF.Exp, scale=-1.0)
                nc.scalar.activation(ls1, s1, AF.Ln)
                nc.scalar.activation(r1s, ls1, AF.Exp, scale=-1.0)
                fill(4)
                term1 = t_("term1")
                nc.vector.tensor_tensor(term1, rho4, r_c, OP.mult)
                q2, q1 = t_("q2"), t_("q1")
                eng("q2").tensor_tensor(q2, d2, r2s, OP.mult)
                eng("q1").tensor_tensor(q1, d1, r1s, OP.mult)
                at2, at1 = t_("at2"), t_("at1")
                nc.scalar.activation(at2, q2, AF.Arctan)
                nc.scalar.activation(at1, q1, AF.Arctan)
                dat = t_("dat")
                eng("dat").tensor_tensor(dat, at2, at1, OP.subtract)
                vv = t_("vv")
                nc.scalar.activation(vv, dat, AF.Square, scale=2.0 / PI)
                fill(8)
                den0 = t_("den0")
                nc.vector.tensor_tensor(den0, vv, iou, OP.subtract)
                lnden, rden, v2 = t_("lnden"), t_("rden"), t_("v2")
                nc.scalar.activation(lnden, den0, AF.Ln, bias=bias_ap(1.0 + EPS))
                nc.scalar.activation(rden, lnden, AF.Exp, scale=-1.0)
                nc.scalar.activation(v2, vv, AF.Square)
                term2, s12, z = t_("term2"), t_("s12"), t_("z")
                eng("term2").tensor_tensor(term2, v2, rden, OP.mult)
                eng("s12").tensor_tensor(s12, term1, term2, OP.add)
                fill(7)
                nc.vector.scalar_tensor_tensor(z, iou, -1.0, s12, OP.mult, OP.add)

                om2, lnsw, sw = t_("om2"), t_("lnsw"), t_("sw")
                nc.scalar.activation(om2, z, AF.Square, bias=bias_ap(1.0))
                nc.scalar.activation(lnsw, a2t, AF.Ln, bias=bias_ap(1e-7))
                nc.scalar.activation(sw, lnsw, AF.Exp, scale=-1.0)
                om3, baset = t_("om3"), t_("baset")
                nc.vector.scalar_tensor_tensor(om3, z, 1.0, om2, OP.add, OP.mult)
                nc.vector.scalar_tensor_tensor(
                    baset, om3, 0.0, sw, OP.add, OP.mult,
                    accum_out=acc_sb[:, n : n + 1],
                )
                fill(32)  # any remaining planes

                # ---- diagonal-lattice matmuls: 8 columns per matmul ----
                # psum[(i,ts_y),(j,ts_x)] (row 8i+ts_y, col 8j+ts_x) sums
                # ohy_t[k,i]*ohx_t'[k,j]; cells with ts_y==ts_x=tr are the true
                # per-column outer products (accumulated over the tile), the
                # rest is cross-column garbage the host never reads.
                # Copy the PREVIOUS tile's psum now -- by this point its
                # matmuls have long finished, so the copy doesn't head-of-line
                # block the DVE queue the way a same-tile copy would.
                if n > 0:
                    nc.vector.tensor_copy(
                        hist_sb[:, 128 * (n - 1) : 128 * n], ps_prev[:])
                oy8 = ohy.rearrange("p (g m) -> p g m", m=128)
                ox8 = ohx.rearrange("p (g m) -> p g m", m=128)
                ps = psp.tile([128, 128], F32, tag="ps", name="ps")
                n_mm = T // 8
                for g in range(n_mm):
                    nc.tensor.matmul(
                        ps[:], oy8[:, g], ox8[:, g],
                        start=(g == 0), stop=(g == n_mm - 1),
                    )
                ps_prev = ps

            nc.vector.tensor_copy(
                hist_sb[:, 128 * (n_tiles - 1) :], ps_prev[:])
            nc.sync.dma_start(hist_d.ap(), hist_sb[:])
            nc.sync.dma_start(acc_d.ap(), acc_sb[:])

    nc.compile()
    return nc


_CACHE = {}
RUN_KW = {}
LAST_RESULT = None


def _get_program(NB, T=512, Tc=None):
    key = (NB, T)
    if key not in _CACHE:
        _CACHE[key] = build_nc(NB, T=T)
    return _CACHE[key]


def _trick16(v):
    """Replicate device magic-number binning exactly (f32 IEEE RNE).
    Returns (z, h): z = f32(16v + 0.5), h = trickfloor = RNE(z+M)-(M+1)."""
    z = (v * np.float32(16.0) + np.float32(0.5)).astype(np.float32)
    t1 = (z + np.float32(MAGIC)).astype(np.float32)
    h = (t1 - np.float32(MAGIC + 1.0)).astype(np.float32)
    return z, h


def _decode_hists(hist_list, targ, n_shard, T):
    """Decode per-core packed histograms.

    hist_list[c] is [128, 128*n_tiles] f64: for tile n, stream tr (= col%8),
    cell (8*hy+tr, 128*n + 8*hx+tr) holds base-64 radix-packed counts:
    digit L = px+2py counts boxes in sub-bin (2hy+py, 2hx+px). Cells with
    row%8 != col%8 are cross-column garbage and are skipped."""
    n_tiles = hist_list[0].shape[1] // 128
    x, y = targ[:, 0], targ[:, 1]
    zx, hxf = _trick16(x)
    zy, hyf = _trick16(y)
    # device: s = zx - hxf (exact); r = relu(s - 1); px = min(r * 2^24, 1)
    # i.e. px = [zx - hxf > 1] (strict), all steps f32-exact.
    px = ((zx - hxf).astype(np.float32) > np.float32(1.0)).astype(np.int64)
    py = ((zy - hyf).astype(np.float32) > np.float32(1.0)).astype(np.int64)
    hx = hxf.astype(np.int64)
    hy = hyf.astype(np.int64)
    gx_f = np.floor((x * np.float32(32.0)).astype(np.float32)).astype(np.int64)
    gy_f = np.floor((y * np.float32(32.0)).astype(np.float32)).astype(np.int64)
    inrange = (hx >= 0) & (hx < 16) & (hy >= 0) & (hy < 16)
    clean = (2 * hx + px == gx_f) & (2 * hy + py == gy_f) & inrange

    hist = np.zeros((GRID, GRID), dtype=np.float64)
    for i in np.nonzero(~clean)[0]:
        c = i // n_shard
        pos = i - c * n_shard
        n = pos // (128 * T)
        tr = (pos % T) % 8
        if inrange[i]:
            hist_list[c][8 * hy[i] + tr, 128 * n + 8 * hx[i] + tr] -= \
                64.0 ** (px[i] + 2 * py[i])
        hist[gy_f[i], gx_f[i]] += 1.0
    sel = np.arange(16) * 8
    for Hc in hist_list:
        for n in range(n_tiles):
            Q = Hc[:, 128 * n : 128 * (n + 1)]
            for tr in range(8):
                P = Q[np.ix_(sel + tr, sel + tr)]
                n0 = P % 64.0
                r = np.floor(P / 64.0)
                n1 = r % 64.0
                r = np.floor(r / 64.0)
                n2 = r % 64.0
                n3 = np.floor(r / 64.0)
                hist[0::2, 0::2] += n0
                hist[0::2, 1::2] += n1
                hist[1::2, 0::2] += n2
                hist[1::2, 1::2] += n3
    return hist


def kernel(pred_boxes: np.ndarray, target_boxes: np.ndarray) -> np.ndarray:
    N = pred_boxes.shape[0]
    assert N % N_CORES == 0
    n_shard = N // N_CORES
    NB = NB_CORE if N == N_TOTAL else n_shard
    pad = NB - n_shard
    assert pad >= 0

    pred = np.ascontiguousarray(pred_boxes, dtype=np.float32)
    targ = np.ascontiguousarray(target_boxes, dtype=np.float32)

    in_maps = []
    padcol = np.array(PAD_BOX, dtype=np.float32)[:, None].repeat(pad, 1) if pad else None
    for c in range(N_CORES):
        ps = pred[c * n_shard : (c + 1) * n_shard].T  # [4, n_shard] planar
        ts = targ[c * n_shard : (c + 1) * n_shard].T
        if pad:
            ps = np.concatenate([ps, padcol], 1)
            ts = np.concatenate([ts, padcol], 1)
        in_maps.append({"pred_boxes": np.ascontiguousarray(ps),
                        "target_boxes": np.ascontiguousarray(ts)})

    nc = _get_program(NB, 512)
    res = bass_utils.run_bass_kernel_spmd(
        nc, in_maps, core_ids=list(range(N_CORES)), **RUN_KW
    )
    global LAST_RESULT
    LAST_RESULT = res

    base_sum = 0.0
    hists = []
    for r in res.results:
        base_sum += float(r["acc_out"].astype(np.float64).sum())
        hists.append(r["hist_out"].astype(np.float64))
    hist = _decode_hists(hists, targ, n_shard, 512)
    assert hist.sum() == N, (hist.sum(), N)
    mean_base = base_sum / N
    max_h = hist.max()
    result = mean_base * (1.0 + ALPHA * (N / (GRID * GRID)) / max_h)
    return np.float32(result)


# revision 26
# speedup vs baseline: 1.0128x; 1.0128x over previous
"""DOSAConLoss Trainium2 kernel.

result = mean(base) * mean(1 + ALPHA * density)
       = mean(base) * (1 + ALPHA * (N/1024) / max_hist)

since sum(hist) == N exactly (every box center lands in one bin).

Per core (8-way data parallel over N): compute
  - per-partition partial sums of base  (acc_out [128, n_tiles])
  - radix-packed 32x32 histogram of target box centers (hist_out)
Host combines: sums acc, decodes + sums hists, applies the scalar formula.

Inputs are staged PLANAR ([4, NB]: x,y,w,h planes) so every DVE read is
dense (the original xywh interleave costs ~2x on strided reads).

Math rewrite (validated vs reference in fp64/fp32):
  dx=x1-x2, W=w1+w2, dW=w1-w2 (same for y/h)
  iw4 = relu(W - max(|2dx|,|dW|)) = 2*iw ; inter4 = iw4*ih4 = 4*inter
  u = a1+a2 - inter4/4 (+eps)  ; iou = inter4 * 0.25/(u+eps)
  cw2 = W + mx = 2*cw ; c24 = cw2^2+ch2^2 = 4*c2 ; rho4 = (2dx)^2+(2dy)^2
  rho2/c2 == rho4/c24
  atan(w/h) = pi/4 + atan((w-h)/(w+h))  [exact identity, arg in (-1,1)]
    -> theta2-theta1 = atan(q2)-atan(q1), q=(w-h)/(w+h)
  v = ((th2-th1)*2/pi)^2 ; ciou = iou - rho4/c24 - v^2/(v-iou+1+eps)
  base = (1-ciou)^3 / (w2*h2 + 1e-7)
Reciprocals via exp(-ln(x)) (ACT Reciprocal is disallowed in bass).

Histogram (radix-packed 2:1 on both axes, diagonal-block matmuls):
  hx = trickfloor(16x) via magic-number RNE rounding; px = [16x+0.5-1 >= hx]
  (x-bin = 2*hx+px), same for y. Weights wx = 64^px, wy = 4096^py.
  One-hot planes (plane-major, dense DVE writes):
    ohx[p, i*T+t] = [hx[p,t]==i]*wx ; ohy[p, m*T+t] = [hy==m]*wy
  Eight box-columns merge into ONE [128,128]x[128,128] matmul: operand APs
  enumerate free dims as (ts=t%8 inner, bin outer), so psum[(tr,m),(ts,i)]
  sums ohy_t[k,m]*ohx_t[k,i] -- the 8 diagonal 16x16 blocks (tr==ts) are the
  true per-column outer products (accumulated over the tile via PSUM); the
  off-diagonal cross-column blocks are garbage and never read. 64 matmuls
  per tile instead of 512 LDWEIGHTS+MATMUL pairs per tile.
  Each psum cell accumulates 64^(px+2py) per box: base-64 digits = counts of
  the four (py,px) sub-bins; digit max ~23 < 64 for uniform inputs.
  Host decodes digits, exactly relocating the few fp-tie boxes where the
  device trick-bin differs from floor.
"""

import numpy as np

import concourse.bass as bass
import concourse.bacc as bacc
import concourse.mybir as mybir
import concourse.tile as tile
from concourse import bass_utils

# The act-table-load chooser picks the first set containing each function,
# which puts Ln in `natural_log` and Exp in `exp_and_others`, forcing a
# ~2.7us table switch at every Ln->Exp pair (we use exp(-ln(x)) for all
# reciprocals). Hide Ln/Exp from the single-function sets so the chooser
# lands on `natural_log_exp_and_others` (set ids keep their act_info.json
# positions; only membership is masked).
_orig_get_act_tables = bacc.get_activation_tables


def _patched_get_act_tables(arch):
    t = {k: set(v) for k, v in _orig_get_act_tables(arch).items()}
    t.get("natural_log", set()).discard(mybir.ActivationFunctionType.Ln)
    t.get("exp_and_others", set()).discard(mybir.ActivationFunctionType.Exp)
    t.get("exp_and_friends", set()).discard(mybir.ActivationFunctionType.Exp)
    return t


bacc.get_activation_tables = _patched_get_act_tables

F32 = mybir.dt.float32
BF16 = mybir.dt.bfloat16
AF = mybir.ActivationFunctionType
OP = mybir.AluOpType

GRID = 32
ALPHA = 1.5
EPS = 1e-7
PI = float(np.pi)
MAGIC = float(2 ** 23)

N_CORES = 8
N_TOTAL = 4_000_000
NB_CORE = 524_288            # padded boxes per core: 128 * 4096
# pad boxes: pred == targ -> base ~1e-21; x=y=0 -> trick-bin -1 -> never
# deposited in the histogram (no host fixup needed).
PAD_BOX = (0.0, 0.0, 1.0, 1.0)

# GPSIMD offload set for 2-input tensor_tensor ops (tune via profile)
# (POOL TensorTensor float ops: only add/subtract/mult are ISA-legal)
GPS_OPS = {"asum", "cw2", "ch2", "c24", "rho4", "s2", "s1"}


def build_nc(NB, T=512, gps=True):
    """Build the per-core Bass program. NB must equal n_tiles*128*T."""
    n_tiles = NB // (128 * T)
    assert NB == n_tiles * 128 * T
    assert T % 8 == 0

    nc = bacc.Bacc("TRN2", target_bir_lowering=False, debug=False)
    pred_d = nc.dram_tensor("pred_boxes", [4, NB], F32, kind="ExternalInput")
    targ_d = nc.dram_tensor("target_boxes", [4, NB], F32, kind="ExternalInput")
    acc_d = nc.dram_tensor("acc_out", [128, n_tiles], F32, kind="ExternalOutput")
    hist_d = nc.dram_tensor("hist_out", [128, 128 * n_tiles], F32, kind="ExternalOutput")

    # planar: channel c of tile n -> [128, T] dense
    pred_v = pred_d.ap().rearrange("c (n p t) -> n p c t", p=128, t=T)
    targ_v = targ_d.ap().rearrange("c (n p t) -> n p c t", p=128, t=T)

    def eng(name):
        return nc.gpsimd if (gps and name in GPS_OPS) else nc.vector

    with tile.TileContext(nc) as tc:
        with (
            tc.tile_pool(name="inp", bufs=2) as inp,
            tc.tile_pool(name="tmp", bufs=2) as tmp,
            tc.tile_pool(name="ohp", bufs=2) as ohp,
            tc.tile_pool(name="cst", bufs=1) as cst,
            tc.tile_pool(name="psp", bufs=2, space="PSUM") as psp,
        ):
            bias_tiles = {}

            def bias_ap(val):
                if val not in bias_tiles:
                    t = cst.tile([128, 1], F32, name=f"bias{len(bias_tiles)}")
                    nc.vector.memset(t[:], val)
                    bias_tiles[val] = t[:]
                return bias_tiles[val]

            acc_sb = cst.tile([128, n_tiles], F32)
            hist_sb = cst.tile([128, 128 * n_tiles], F32)

            # Temp slot allocator: long-lived temps get dedicated tags;
            # short-lived ones rotate through NGEN generic tags (bufs=2 each,
            # Tile inserts WAR deps on slot reuse). Max temp lifetime must be
            # < 2*NGEN generic allocations.
            NGEN = 12
            DEDICATED = {"W", "H", "a2t", "iou", "term1", "vv", "d2", "d1"}
            gen_counter = [0]

            for n in range(n_tiles):
                pt = inp.tile([128, 4 * T], F32, tag="pred")
                tt = inp.tile([128, 4 * T], F32, tag="targ")
                p3 = pt.rearrange("p (c t) -> p c t", c=4)
                t3 = tt.rearrange("p (c t) -> p c t", c=4)
                nc.sync.dma_start(p3, pred_v[n])
                nc.sync.dma_start(t3, targ_v[n])
                x1, y1, w1, h1 = p3[:, 0], p3[:, 1], p3[:, 2], p3[:, 3]
                x2, y2, w2, h2 = t3[:, 0], t3[:, 1], t3[:, 2], t3[:, 3]

                def t_(tag, dt=F32):
                    if tag in DEDICATED:
                        return tmp.tile([128, T], dt, tag=tag, name=tag)[:]
                    i = gen_counter[0] % NGEN
                    gen_counter[0] += 1
                    return tmp.tile([128, T], dt, tag=f"g{i}", name=tag)[:]

                # ---- histogram prep first (depends only on targ x/y) ----
                # hx = trickfloor(16x): t1 = RNE(16x + 0.5 + 2^23);
                # hx = t1 - (2^23+1). Pair parity px = [16x+0.5 - hx > 1] via
                # exact arithmetic (relational-op1 stt runs ~2.5x slower):
                # r = relu(s-1), px = min(r*2^24, 1) -- all steps f32-exact and
                # host-replicable. Combined weight wxy = 64^px * 4096^py rides
                # on the y-side one-hot so the x-side is single-input is_equal.
                # Ties / out-of-range corrected host-side (see _decode_hists).
                zx, zy, hxf, hyf = t_("zx"), t_("zy"), t_("hxf"), t_("hyf")
                sx, sy, rx, ry = t_("sx"), t_("sy"), t_("rx"), t_("ry")
                hx = tmp.tile([128, T], BF16, tag="hx", name="hx")[:]
                px = tmp.tile([128, T], BF16, tag="px", name="px")[:]
                wx = tmp.tile([128, T], BF16, tag="wx", name="wx")[:]
                hy = tmp.tile([128, T], BF16, tag="hy", name="hy")[:]
                py = tmp.tile([128, T], BF16, tag="py", name="py")[:]
                wy = tmp.tile([128, T], BF16, tag="wy", name="wy")[:]
                wxy = tmp.tile([128, T], BF16, tag="wxy", name="wxy")[:]
                nc.vector.tensor_scalar(zx, x2, 16.0, 0.5, OP.mult, OP.add)
                nc.vector.tensor_scalar(hxf, zx, MAGIC, MAGIC + 1.0, OP.add, OP.subtract)
                nc.vector.tensor_scalar(hx, zx, MAGIC, MAGIC + 1.0, OP.add, OP.subtract)
                nc.vector.tensor_tensor(sx, zx, hxf, OP.subtract)
                nc.scalar.activation(rx, sx, AF.Relu, bias=bias_ap(-1.0))
                nc.vector.tensor_scalar(zy, y2, 16.0, 0.5, OP.mult, OP.add)
                nc.vector.tensor_scalar(hyf, zy, MAGIC, MAGIC + 1.0, OP.add, OP.subtract)
                nc.vector.tensor_scalar(hy, zy, MAGIC, MAGIC + 1.0, OP.add, OP.subtract)
                nc.vector.tensor_tensor(sy, zy, hyf, OP.subtract)
                nc.scalar.activation(ry, sy, AF.Relu, bias=bias_ap(-1.0))

                # ---- first-level geometry (DVE, dep-free) ----
                dx, dy = t_("dx"), t_("dy")
                W, dW, H, dH = t_("W"), t_("dW"), t_("H"), t_("dH")
                nc.vector.tensor_tensor(dx, x1, x2, OP.subtract)
                nc.vector.tensor_tensor(dy, y1, y2, OP.subtract)
                nc.vector.tensor_tensor(W, w1, w2, OP.add)
                nc.vector.tensor_tensor(dW, w1, w2, OP.subtract)
                nc.vector.tensor_tensor(H, h1, h2, OP.add)
                nc.vector.tensor_tensor(dH, h1, h2, OP.subtract)
                a2t, a1t, asum = t_("a2t"), t_("a1t"), t_("asum")
                nc.vector.tensor_tensor(a2t, w2, h2, OP.mult)
                nc.vector.tensor_tensor(a1t, w1, h1, OP.mult)
                d2, d1 = t_("d2"), t_("d1")
                nc.vector.tensor_tensor(d2, w2, h2, OP.subtract)
                nc.vector.tensor_tensor(d1, w1, h1, OP.subtract)
                s2, s1 = t_("s2"), t_("s1")
                eng("asum").tensor_tensor(asum, a1t, a2t, OP.add)
                eng("s2").tensor_tensor(s2, w2, h2, OP.add)
                eng("s1").tensor_tensor(s1, w1, h1, OP.add)

                adx, ady, adW, adH = t_("adx"), t_("ady"), t_("adW"), t_("adH")
                nc.scalar.activation(adx, dx, AF.Abs, scale=2.0)
                nc.scalar.activation(ady, dy, AF.Abs, scale=2.0)
                nc.scalar.activation(adW, dW, AF.Abs)
                nc.scalar.activation(adH, dH, AF.Abs)
                ls2, r2s, ls1, r1s = t_("ls2"), t_("r2s"), t_("ls1"), t_("r1s")
                nc.scalar.activation(ls2, s2, AF.Ln)
                nc.scalar.activation(r2s, ls2, AF.Exp, scale=-1.0)
                nc.scalar.activation(ls1, s1, AF.Ln)
                nc.scalar.activation(r1s, ls1, AF.Exp, scale=-1.0)

                nc.vector.tensor_scalar(px, rx, 16777216.0, 1.0, OP.mult, OP.min)
                nc.vector.tensor_scalar(py, ry, 16777216.0, 1.0, OP.mult, OP.min)
                nc.scalar.activation(wx, px, AF.Copy, scale=63.0, bias=1.0)
                nc.scalar.activation(wy, py, AF.Copy, scale=4095.0, bias=1.0)
                nc.vector.tensor_tensor(wxy, wx, wy, OP.mult)

                # ---- one-hot build, GROUP-BLOCKED layout ----
                # ohx[p, g*128 + i*8 + ts] = [hx[p, 8g+ts]==i] (plain 0/1);
                # ohy[...] = [hy==i]*wxy. Per bin the write AP is
                # (g: stride 128) x (ts: 8 contiguous) -- 16B runs, near-dense
                # for the DVE write port -- while each group's matmul operand
                # [128, 128] is fully contiguous. The 32 plane ops are emitted
                # interleaved into the main chain as DVE filler in front of
                # cross-engine dependencies (ACT/GPS results arrive ~1-5us
                # late; DVE queues are strict FIFO so independent work must
                # sit BEFORE the dependent op to hide the latency).
                ohx = ohp.tile([128, 16 * T], BF16, tag="ohx", name="ohx")
                ohy = ohp.tile([128, 16 * T], BF16, tag="ohy", name="ohy")
                vx = ohx.rearrange("p (g i ts) -> p i g ts", i=16, ts=8)
                vy = ohy.rearrange("p (g i ts) -> p i g ts", i=16, ts=8)
                hx8 = hx.rearrange("p (g ts) -> p g ts", ts=8)
                hy8 = hy.rearrange("p (g ts) -> p g ts", ts=8)
                wxy8 = wxy.rearrange("p (g ts) -> p g ts", ts=8)
                planes = []
                for i in range(16):
                    planes.append((vx, i, None))
                    planes.append((vy, i, wxy8))
                plane_i = [0]

                def fill(k):
                    for _ in range(k):
                        if plane_i[0] >= len(planes):
                            return
                        v, i, w = planes[plane_i[0]]
                        plane_i[0] += 1
                        if w is None:
                            nc.vector.tensor_scalar(
                                v[:, i], hx8, float(i), None, OP.is_equal)
                        else:
                            nc.vector.scalar_tensor_tensor(
                                v[:, i], hy8, float(i), w, OP.is_equal, OP.mult)

                fill(6)   # x-planes while ACT computes the abs block
                mx, my = t_("mx"), t_("my")
                nc.vector.tensor_tensor(mx, adx, adW, OP.max)
                nc.vector.tensor_tensor(my, ady, adH, OP.max)

                iw4, ih4, ihc, inter4 = t_("iw4"), t_("ih4"), t_("ihc"), t_("inter4")
                nc.vector.scalar_tensor_tensor(iw4, mx, -1.0, W, OP.mult, OP.add)
                nc.vector.scalar_tensor_tensor(ih4, my, -1.0, H, OP.mult, OP.add)
                nc.vector.tensor_scalar(ihc, ih4, 0.0, None, OP.max)
                cw2, ch2 = t_("cw2"), t_("ch2")
                eng("cw2").tensor_tensor(cw2, W, mx, OP.add)
                eng("ch2").tensor_tensor(ch2, H, my, OP.add)
                fill(2)
                nc.vector.scalar_tensor_tensor(inter4, iw4, 0.0, ihc, OP.max, OP.mult)
                u = t_("u")
                nc.vector.scalar_tensor_tensor(u, inter4, -0.25, asum, OP.mult, OP.add)
                lnu, r_u = t_("lnu"), t_("r_u")
                nc.scalar.activation(lnu, u, AF.Ln, scale=4.0, bias=bias_ap(4 * EPS))
                nc.scalar.activation(r_u, lnu, AF.Exp, scale=-1.0)
                scw, sch, sdx, sdy = t_("scw"), t_("sch"), t_("sdx"), t_("sdy")
                nc.scalar.activation(scw, cw2, AF.Square)
                nc.scalar.activation(sch, ch2, AF.Square)
                nc.scalar.activation(sdx, adx, AF.Square)
                nc.scalar.activation(sdy, ady, AF.Square)
                c24, rho4 = t_("c24"), t_("rho4")
                eng("c24").tensor_tensor(c24, scw, sch, OP.add)
                eng("rho4").tensor_tensor(rho4, sdx, sdy, OP.add)
                lnc, r_c = t_("lnc"), t_("r_c")
                nc.scalar.activation(lnc, c24, AF.Ln, bias=bias_ap(4 * EPS))
                nc.scalar.activation(r_c, lnc, AF.Exp, scale=-1.0)
                # q/atan chain: r2s/r1s were computed early in the ACT queue
                q2, q1 = t_("q2"), t_("q1")
                nc.vector.tensor_tensor(q2, d2, r2s, OP.mult)
                nc.vector.tensor_tensor(q1, d1, r1s, OP.mult)
                at2, at1 = t_("at2"), t_("at1")
                nc.scalar.activation(at2, q2, AF.Arctan)
                nc.scalar.activation(at1, q1, AF.Arctan)
                fill(3)
                iou = t_("iou")
                nc.vector.tensor_tensor(iou, inter4, r_u, OP.mult)
                dat = t_("dat")
                nc.vector.tensor_tensor(dat, at2, at1, OP.subtract)
                vv = t_("vv")
                nc.scalar.activation(vv, dat, AF.Square, scale=2.0 / PI)
                fill(5)
                term1 = t_("term1")
                nc.vector.tensor_tensor(term1, rho4, r_c, OP.mult)
                fill(4)
                den0 = t_("den0")
                nc.vector.tensor_tensor(den0, vv, iou, OP.subtract)
                lnden, rden, v2 = t_("lnden"), t_("rden"), t_("v2")
                nc.scalar.activation(lnden, den0, AF.Ln, bias=bias_ap(1.0 + EPS))
                nc.scalar.activation(rden, lnden, AF.Exp, scale=-1.0)
                nc.scalar.activation(v2, vv, AF.Square)
                fill(6)
                term2, s12, z = t_("term2"), t_("s12"), t_("z")
                nc.vector.tensor_tensor(term2, v2, rden, OP.mult)
                nc.vector.tensor_tensor(s12, term1, term2, OP.add)
                fill(6)
                nc.vector.scalar_tensor_tensor(z, iou, -1.0, s12, OP.mult, OP.add)

                om2, lnsw, sw = t_("om2"), t_("lnsw"), t_("sw")
                nc.scalar.activation(om2, z, AF.Square, bias=bias_ap(1.0))
                nc.scalar.activation(lnsw, a2t, AF.Ln, bias=bias_ap(1e-7))
                nc.scalar.activation(sw, lnsw, AF.Exp, scale=-1.0)
                om3, baset = t_("om3"), t_("baset")
                nc.vector.scalar_tensor_tensor(om3, z, 1.0, om2, OP.add, OP.mult)
                nc.vector.scalar_tensor_tensor(
                    baset, om3, 0.0, sw, OP.add, OP.mult,
                    accum_out=acc_sb[:, n : n + 1],
                )
                fill(32)  # any remaining planes

                # ---- diagonal-lattice matmuls: 8 columns per matmul ----
                # psum[(i,ts_y),(j,ts_x)] (row 8i+ts_y, col 8j+ts_x) sums
                # ohy_t[k,i]*ohx_t'[k,j]; cells with ts_y==ts_x=tr are the true
                # per-column outer products (accumulated over the tile), the
                # rest is cross-column garbage the host never reads.
                # Copy the PREVIOUS tile's psum now -- by this point its
                # matmuls have long finished, so the copy doesn't head-of-line
                # block the DVE queue the way a same-tile copy would.
                if n > 0:
                    nc.vector.tensor_copy(
                        hist_sb[:, 128 * (n - 1) : 128 * n], ps_prev[:])
                oy8 = ohy.rearrange("p (g m) -> p g m", m=128)
                ox8 = ohx.rearrange("p (g m) -> p g m", m=128)
                ps = psp.tile([128, 128], F32, tag="ps", name="ps")
                n_mm = T // 8
                for g in range(n_mm):
                    nc.tensor.matmul(
                        ps[:], oy8[:, g], ox8[:, g],
                        start=(g == 0), stop=(g == n_mm - 1),
                    )
                ps_prev = ps

            nc.vector.tensor_copy(
                hist_sb[:, 128 * (n_tiles - 1) :], ps_prev[:])
            nc.sync.dma_start(hist_d.ap(), hist_sb[:])
            nc.sync.dma_start(acc_d.ap(), acc_sb[:])

    nc.compile()
    return nc


_CACHE = {}
RUN_KW = {}
LAST_RESULT = None


def _get_program(NB, T=512, Tc=None):
    key = (NB, T)
    if key not in _CACHE:
        _CACHE[key] = build_nc(NB, T=T)
    return _CACHE[key]


def _trick16(v):
    """Replicate device magic-number binning exactly (f32 IEEE RNE).
    Returns (z, h): z = f32(16v + 0.5), h = trickfloor = RNE(z+M)-(M+1)."""
    z = (v * np.float32(16.0) + np.float32(0.5)).astype(np.float32)
    t1 = (z + np.float32(MAGIC)).astype(np.float32)
    h = (t1 - np.float32(MAGIC + 1.0)).astype(np.float32)
    return z, h


def _decode_hists(hist_list, targ, n_shard, T):
    """Decode per-core packed histograms.

    hist_list[c] is [128, 128*n_tiles] f64: for tile n, stream tr (= col%8),
    cell (8*hy+tr, 128*n + 8*hx+tr) holds base-64 radix-packed counts:
    digit L = px+2py counts boxes in sub-bin (2hy+py, 2hx+px). Cells with
    row%8 != col%8 are cross-column garbage and are skipped."""
    n_tiles = hist_list[0].shape[1] // 128
    x, y = targ[:, 0], targ[:, 1]
    zx, hxf = _trick16(x)
    zy, hyf = _trick16(y)
    # device: s = zx - hxf (exact); r = relu(s - 1); px = min(r * 2^24, 1)
    # i.e. px = [zx - hxf > 1] (strict), all steps f32-exact.
    px = ((zx - hxf).astype(np.float32) > np.float32(1.0)).astype(np.int64)
    py = ((zy - hyf).astype(np.float32) > np.float32(1.0)).astype(np.int64)
    hx = hxf.astype(np.int64)
    hy = hyf.astype(np.int64)
    gx_f = np.floor((x * np.float32(32.0)).astype(np.float32)).astype(np.int64)
    gy_f = np.floor((y * np.float32(32.0)).astype(np.float32)).astype(np.int64)
    inrange = (hx >= 0) & (hx < 16) & (hy >= 0) & (hy < 16)
    clean = (2 * hx + px == gx_f) & (2 * hy + py == gy_f) & inrange

    hist = np.zeros((GRID, GRID), dtype=np.float64)
    for i in np.nonzero(~clean)[0]:
        c = i // n_shard
        pos = i - c * n_shard
        n = pos // (128 * T)
        tr = (pos % T) % 8
        if inrange[i]:
            hist_list[c][8 * hy[i] + tr, 128 * n + 8 * hx[i] + tr] -= \
                64.0 ** (px[i] + 2 * py[i])
        hist[gy_f[i], gx_f[i]] += 1.0
    sel = np.arange(16) * 8
    for Hc in hist_list:
        for n in range(n_tiles):
            Q = Hc[:, 128 * n : 128 * (n + 1)]
            for tr in range(8):
                P = Q[np.ix_(sel + tr, sel + tr)]
                n0 = P % 64.0
                r = np.floor(P / 64.0)
                n1 = r % 64.0
                r = np.floor(r / 64.0)
                n2 = r % 64.0
                n3 = np.floor(r / 64.0)
                hist[0::2, 0::2] += n0
                hist[0::2, 1::2] += n1
                hist[1::2, 0::2] += n2
                hist[1::2, 1::2] += n3
    return hist


def kernel(pred_boxes: np.ndarray, target_boxes: np.ndarray) -> np.ndarray:
    N = pred_boxes.shape[0]
    assert N % N_CORES == 0
    n_shard = N // N_CORES
    NB = NB_CORE if N == N_TOTAL else n_shard
    pad = NB - n_shard
    assert pad >= 0

    pred = np.ascontiguousarray(pred_boxes, dtype=np.float32)
    targ = np.ascontiguousarray(target_boxes, dtype=np.float32)

    in_maps = []
    padcol = np.array(PAD_BOX, dtype=np.float32)[:, None].repeat(pad, 1) if pad else None
    for c in range(N_CORES):
        ps = pred[c * n_shard : (c + 1) * n_shard].T  # [4, n_shard] planar
        ts = targ[c * n_shard : (c + 1) * n_shard].T
        if pad:
            ps = np.concatenate([ps, padcol], 1)
            ts = np.concatenate([ts, padcol], 1)
        in_maps.append({"pred_boxes": np.ascontiguousarray(ps),
                        "target_boxes": np.ascontiguousarray(ts)})

    nc = _get_program(NB, 512)
    res = bass_utils.run_bass_kernel_spmd(
        nc, in_maps, core_ids=list(range(N_CORES)), **RUN_KW
    )
    global LAST_RESULT
    LAST_RESULT = res

    base_sum = 0.0
    hists = []
    for r in res.results:
        base_sum += float(r["acc_out"].astype(np.float64).sum())
        hists.append(r["hist_out"].astype(np.float64))
    hist = _decode_hists(hists, targ, n_shard, 512)
    assert hist.sum() == N, (hist.sum(), N)
    mean_base = base_sum / N
    max_h = hist.max()
    result = mean_base * (1.0 + ALPHA * (N / (GRID * GRID)) / max_h)
    return np.float32(result)
